# revision 28
# baseline (speedup 1.0000x reference)
"""Trainium2 Bass kernel for a 2-layer DenseGCN encoder with mean+max readout.

Reference (per graph b; B=256 graphs, N=256 nodes, F=128 features):
    A  = adj with diagonal set to 1.0                  (host-side prep)
    d  = rowsum(A) ** -0.5        (rowsum >= 1: diag=1, offdiag >= 0)
    An = d[:,None] * A * d[None,:]   (S A S, symmetric; S = diag(d))
    H1 = An @ X @ W1 + b1
    H2 = An @ H1 @ W2 + b2
    out = concat([mean_n(H2), max_n(H2)]) @ Wr + br

Device mapping, v7. All matmuls use the RAW (unnormalized) adjacency as the
moving operand; the four diag-scales (S per side per layer) ride on small
tensors instead, so no scaled copy of A is ever materialized:
    xs   = S X                  (one TT per 4-graph group, d broadcast-AP)
    C    = xs^T A               (= X^T S A)                        [PE]
    c_sb = plain copy           (bf16)                             [ACT]
    M1r  = c_sb^T W1            (= A S X W1)                       [PE]
    h1s  = d^2-quads * M1r      (TT; = S^2 M1r = S H1, C2's lhsT)  [DVE]
    C2   = h1s^T A              (= H1^T S A)                       [PE]
    c2_sb= plain copy           (bf16)                             [ACT]
    M2Tr = W2^T c2_sb           (= (A S H1 W2)^T)                  [PE]
    scld = M2Tr * dbc           (TT; = H2^T pre-b2)                [DVE]
    pooled_s[:,g] = accum of act-copy(scld_g);  pooled_m = reduce_max(scld)
    out = pooled_s^T Wr_s + pooled_m^T Wr_m + 1 br_eff^T  (fp32)   [PE]
d comes from: rT = 3D reduce_sum of the adj group (row form) -> act-Rsqrt;
dbc from PE colsums (N=512 per pair, [t,g,n] group layout) -> act-Rsqrt.
b2 and the mean's 1/N are folded into br_eff / Wr_s on the host.

Sharding: data-parallel over the batch dim, 32 graphs per core x 8 cores.
Inputs are cast to bf16 and re-laid out partition-major on the host.
"""

import numpy as np
import ml_dtypes

B, N, F = 256, 256, 128
NCORES = 8
GPC = B // NCORES  # graphs per core
AGSZ = 4  # graphs per adj DMA group
XGSZ = 8  # graphs per x DMA group
NGRP = GPC // AGSZ

_CACHE = {}


def _build_program(with_b1: bool):
    import concourse.bass as bass
    import concourse.mybir as mybir
    import concourse.tile as tile
    from concourse import bacc
    from contextlib import ExitStack

    f32 = mybir.dt.float32
    bf16 = mybir.dt.bfloat16
    MULT = mybir.AluOpType.mult
    ADD = mybir.AluOpType.add
    AX = mybir.AxisListType.X
    COPY = mybir.ActivationFunctionType.Copy
    SQUARE = mybir.ActivationFunctionType.Square

    nc = bacc.Bacc("TRN2", target_bir_lowering=False, debug=False,
                   num_devices=NCORES)

    def act_rsqrt(out, in_):
        # Rsqrt via direct InstActivation: bass's activation() refuses Rsqrt
        # on accuracy-policy grounds (~1e-5 rel here, fine for this kernel).
        eng = nc.scalar
        bias = nc.const_aps.scalar_like(0.0, in_)
        ins = [eng.lower_ap(in_), eng.lower_ap(bias)]
        for arg in (1.0, 0.0):
            ins.append(mybir.ImmediateValue(dtype=f32, value=arg))
        return eng.add_instruction(mybir.InstActivation(
            name=nc.get_next_instruction_name(),
            func=mybir.ActivationFunctionType.Rsqrt,
            ins=ins, outs=[eng.lower_ap(out)]))

    # adj: [128, group, t, g, n] so colsum pairs get 512-wide contiguous rhs
    adjin = nc.dram_tensor("adjin", [128, NGRP, 2, AGSZ, N], bf16,
                           kind="ExternalInput").ap()
    xin = nc.dram_tensor("xin", [128, GPC, 2, F], bf16,
                         kind="ExternalInput").ap()
    cw1 = nc.dram_tensor("cw1", [F, F], bf16, kind="ExternalInput").ap()
    cw2 = nc.dram_tensor("cw2", [F, F], bf16, kind="ExternalInput").ap()
    cwrs = nc.dram_tensor("cwrs", [F, F], f32, kind="ExternalInput").ap()
    cwrm = nc.dram_tensor("cwrm", [F, F], f32, kind="ExternalInput").ap()
    cbr = nc.dram_tensor("cbr", [1, F], f32, kind="ExternalInput").ap()
    cones = nc.dram_tensor("cones", [128, 128], bf16,
                           kind="ExternalInput").ap()
    cones32 = nc.dram_tensor("cones32", [1, GPC], f32,
                             kind="ExternalInput").ap()
    if with_b1:
        cb1 = nc.dram_tensor("cb1", [128, 2 * N], bf16,
                             kind="ExternalInput").ap()
    out_d = nc.dram_tensor("out", [GPC, F], f32, kind="ExternalOutput").ap()

    with tile.TileContext(nc) as tc, ExitStack() as ctx:
        p_const = ctx.enter_context(tc.tile_pool(name="const", bufs=1))
        p_ag = ctx.enter_context(tc.tile_pool(name="ag", bufs=NGRP))
        p_xg = ctx.enter_context(tc.tile_pool(name="xg", bufs=NGRP))
        p_xs = ctx.enter_context(tc.tile_pool(name="xs", bufs=4))
        p_sb = ctx.enter_context(tc.tile_pool(name="sb", bufs=6))
        p_tinyb = ctx.enter_context(tc.tile_pool(name="tinyb", bufs=6))
        p_acc = ctx.enter_context(tc.tile_pool(name="acc", bufs=1))
        ps_s = ctx.enter_context(tc.tile_pool(name="pss", bufs=1, space="PSUM"))
        ps_cc = ctx.enter_context(tc.tile_pool(name="pscc", bufs=4, space="PSUM"))
        ps_m1 = ctx.enter_context(tc.tile_pool(name="psm1", bufs=2, space="PSUM"))
        ps_m2 = ctx.enter_context(tc.tile_pool(name="psm2", bufs=1, space="PSUM"))

        def cload(ap, shape, tag, dt):
            t = p_const.tile(shape, dt, tag=tag, name=tag)
            nc.gpsimd.dma_start(t[:], ap)
            return t

        ones128 = cload(cones, [128, 128], "ones128", bf16)
        w1 = cload(cw1, [F, F], "w1", bf16)
        w2 = cload(cw2, [F, F], "w2", bf16)
        wrs = cload(cwrs, [F, F], "wrs", f32)
        wrm = cload(cwrm, [F, F], "wrm", f32)
        br_row = cload(cbr, [1, F], "br_row", f32)
        ones32 = cload(cones32, [1, GPC], "ones32", f32)
        if with_b1:
            b1bc = cload(cb1, [128, 2 * N], "b1bc", bf16)

        ag_tiles = [None] * NGRP
        xg_tiles = [None] * NGRP

        def load_ag(i):
            t = p_ag.tile([128, AGSZ * 2 * N], bf16, tag="ag", name="ag")
            dst = t[:].rearrange("p (t g n) -> p t g n", t=2, g=AGSZ, n=N)
            nc.sync.dma_start(dst, adjin[:, i])
            ag_tiles[i] = t

        def load_xg(i):
            t = p_xg.tile([128, AGSZ * 2 * F], bf16, tag="xg", name="xg")
            dst = t[:].rearrange("p (g t f) -> p g t f", g=AGSZ, t=2, f=F)
            nc.gpsimd.dma_start(dst, xin[:, i * AGSZ:(i + 1) * AGSZ])
            xg_tiles[i] = t

        for i in range(NGRP):
            load_ag(i)
            load_xg(i)

        pooled_s = p_acc.tile([F, GPC], f32, tag="pooled_s")
        pooled_m = p_acc.tile([F, GPC], f32, tag="pooled_m")

        # ---- per-group state ----
        dT_b = [None] * NGRP   # bf16 [128, (t,g)=8]: d per-partition
        dT2_b = [None] * NGRP  # bf16 [128, 8]: d^2 per-partition
        xs_tiles = [None] * NGRP  # bf16 [128, (g,t,F)=1024]: S X
        state = {}

        def ah(j, q, t):
            # raw adj of graph pair-j + q, chunk t: [128, N]
            agi = (2 * j) // AGSZ
            gg = (2 * j) % AGSZ + q
            return ag_tiles[agi][:, (t * AGSZ + gg) * N:(t * AGSZ + gg + 1) * N]

        def emit_group(agis):
            # rowsum-reduce per ag tile into one shared rT tile, then a
            # single rsqrt/copy/Square chain over all of them
            k = len(agis)
            rT = p_tinyb.tile([128, k * 2 * AGSZ], f32, tag="rT", name="rT")
            for i, agi in enumerate(agis):
                nc.vector.reduce_sum(
                    rT[:, i * 2 * AGSZ:(i + 1) * 2 * AGSZ],
                    ag_tiles[agi][:].rearrange("p (q n) -> p q n",
                                               q=2 * AGSZ, n=N),
                    axis=AX)
            dTf = p_tinyb.tile([128, k * 2 * AGSZ], f32, tag="dTf",
                               name="dTf")
            act_rsqrt(dTf[:], rT[:])
            dTb = p_tinyb.tile([128, k * 2 * AGSZ], bf16, tag="dTb",
                               name="dTb")
            nc.scalar.copy(dTb[:], dTf[:])
            dT2b = p_tinyb.tile([128, k * 2 * AGSZ], bf16, tag="dT2b",
                                name="dT2b")
            nc.scalar.activation(dT2b[:], dTf[:], SQUARE)
            for i, agi in enumerate(agis):
                dT_b[agi] = dTb[:, i * 2 * AGSZ:(i + 1) * 2 * AGSZ]
                dT2_b[agi] = dT2b[:, i * 2 * AGSZ:(i + 1) * 2 * AGSZ]
                emit_xs(agi)
        def emit_xs(agi):
            # xs = S X for the group's 4 graphs (one TT, d broadcast-AP)
            X = xg_tiles[agi]
            xs = p_xs.tile([128, AGSZ * 2 * F], bf16, tag="xs", name="xs")
            in1 = dT_b[agi].rearrange("p (t g) -> p g t", t=2, g=AGSZ) \
                .broadcast_to((128, AGSZ, 2, F))
            nc.vector.tensor_tensor(
                out=xs[:].rearrange("p (g t f) -> p g t f", g=AGSZ, t=2),
                in0=X[:].rearrange("p (g t f) -> p g t f", g=AGSZ, t=2),
                in1=in1, op=MULT)
            xs_tiles[agi] = xs

        def emit_colsum(j):
            # colsums of the raw pair (512-wide rhs) -> dbc = d[n] broadcast
            agi = (2 * j) // AGSZ
            ag = ag_tiles[agi]
            pq = ((2 * j) % AGSZ) // 2
            s_ps = ps_s.tile([128, 2 * N], f32, tag="s", name="s_ps")
            for t in range(2):
                off = (t * AGSZ + pq * 2) * N
                nc.tensor.matmul(s_ps[:], ones128[:],
                                 ag[:, off:off + 2 * N],
                                 start=(t == 0), stop=(t == 1))
            dbc = p_tinyb.tile([128, 2 * N], bf16, tag="dbc", name="dbc")
            act_rsqrt(dbc[:], s_ps[:])
            state[("dbc", j)] = dbc

        def emit_C(j):
            agi = (2 * j) // AGSZ
            xs = xs_tiles[agi]
            c_ps = ps_cc.tile([F, 2 * N], f32, tag="cc", name="c_ps")
            for q in range(2):
                goff = ((2 * j) % AGSZ + q) * 2 * F
                for t in range(2):
                    nc.tensor.matmul(
                        c_ps[:, q * N:(q + 1) * N],
                        xs[:, goff + t * F: goff + (t + 1) * F],
                        ah(j, q, t), start=(t == 0), stop=(t == 1))
            c_sb = p_sb.tile([F, 2 * N], bf16, tag="c_sb", name="c_sb")
            nc.scalar.copy(c_sb[:], c_ps[:])
            state[("c", j)] = c_sb

        def emit_M1(j):
            agi = (2 * j) // AGSZ
            c_sb = state.pop(("c", j))
            m1_ps = ps_m1.tile([128, 2 * N], f32, tag="m1", name="m1_ps")
            for q in range(2):
                for tp in range(2):
                    nc.tensor.matmul(
                        m1_ps[:, (2 * q + tp) * F:(2 * q + tp + 1) * F],
                        c_sb[:, q * N + tp * 128: q * N + tp * 128 + 128],
                        w1[:], start=True, stop=True)
            # h1s = S^2 M1r (C2's lhsT); quads (q,tp) pick d^2 columns
            h1s = p_sb.tile([128, 2 * N], bf16, tag="h1", name="h1")
            g0 = (2 * j) % AGSZ
            in1 = dT2_b[agi].rearrange("p (t g) -> p g t", t=2, g=AGSZ) \
                [:, g0:g0 + 2, :].broadcast_to((128, 2, 2, F))
            if not with_b1:
                nc.vector.tensor_tensor(
                    out=h1s[:].rearrange("p (q tp f) -> p q tp f", q=2, tp=2),
                    in0=m1_ps[:].rearrange("p (q tp f) -> p q tp f", q=2,
                                           tp=2),
                    in1=in1, op=MULT)
            else:
                in1d = dT_b[agi].rearrange("p (t g) -> p g t", t=2, g=AGSZ) \
                    [:, g0:g0 + 2, :].broadcast_to((128, 2, 2, F))
                tmp = p_sb.tile([128, 2 * N], bf16, tag="h1tmp", name="h1tmp")
                nc.vector.tensor_tensor(
                    out=tmp[:].rearrange("p (q tp f) -> p q tp f", q=2, tp=2),
                    in0=m1_ps[:].rearrange("p (q tp f) -> p q tp f", q=2,
                                           tp=2),
                    in1=in1d, op=MULT)  # = S M1r = H1 (pre-bias)
                nc.vector.tensor_tensor(out=tmp[:], in0=tmp[:], in1=b1bc[:],
                                        op=ADD)  # = H1
                nc.vector.tensor_tensor(
                    out=h1s[:].rearrange("p (q tp f) -> p q tp f", q=2, tp=2),
                    in0=tmp[:].rearrange("p (q tp f) -> p q tp f", q=2, tp=2),
                    in1=in1d, op=MULT)  # = S H1
            state[("h1", j)] = h1s

        def emit_C2(j):
            h1s = state.pop(("h1", j))
            c2_ps = ps_cc.tile([F, 2 * N], f32, tag="cc", name="c2_ps")
            for q in range(2):
                for t in range(2):
                    nc.tensor.matmul(
                        c2_ps[:, q * N:(q + 1) * N],
                        h1s[:, (2 * q + t) * F:(2 * q + t + 1) * F],
                        ah(j, q, t), start=(t == 0), stop=(t == 1))
            c2_sb = p_sb.tile([F, 2 * N], bf16, tag="c2_sb", name="c2_sb")
            nc.scalar.copy(c2_sb[:], c2_ps[:])
            state[("c2", j)] = c2_sb

        def emit_M2T(j):
            g0 = 2 * j
            c2_sb = state.pop(("c2", j))
            dbc = state.pop(("dbc", j))
            m2t_ps = ps_m2.tile([F, 2 * N], f32, tag="m2t", name="m2t_ps")
            nc.tensor.matmul(m2t_ps[:], w2[:], c2_sb[:], start=True, stop=True)
            scld = p_sb.tile([F, 2 * N], bf16, tag="scld", name="scld")
            nc.vector.tensor_tensor(out=scld[:], in0=m2t_ps[:], in1=dbc[:],
                                    op=MULT)
            scr = p_tinyb.tile([F, N], bf16, tag="scr", name="scr")
            for q in range(2):
                nc.scalar.activation(
                    scr[:], scld[:, q * N:(q + 1) * N], COPY,
                    accum_out=pooled_s[:, g0 + q:g0 + q + 1])
            nc.vector.reduce_max(
                pooled_m[:, g0:g0 + 2],
                scld[:].rearrange("p (q n) -> p q n", q=2, n=N), axis=AX)

        # ---- software pipeline over pairs ----
        NPAIR = GPC // 2
        emit_group([0])
        emitted = {0}
        for j in range(NPAIR + 3):
            nxt = j // 2 + 1  # group needed soon
            if j < NPAIR and j % 2 == 1 and nxt < NGRP and nxt not in emitted:
                emit_group([nxt])
                emitted.add(nxt)
            if j < NPAIR:
                emit_colsum(j)
                emit_C(j)
            if 0 <= j - 1 < NPAIR:
                emit_M1(j - 1)
            if 0 <= j - 2 < NPAIR:
                emit_C2(j - 2)
            if 0 <= j - 3 < NPAIR:
                emit_M2T(j - 3)

        # readout: out = pooled_s^T Wr_s + pooled_m^T Wr_m + 1 br^T (fp32)
        out_ps = ps_m2.tile([GPC, F], f32, tag="m2t", name="out_ps")
        nc.tensor.matmul(out_ps[:], pooled_s[:], wrs[:], start=True, stop=False)
        nc.tensor.matmul(out_ps[:], pooled_m[:], wrm[:], start=False,
                         stop=False)
        nc.tensor.matmul(out_ps[:], ones32[:], br_row[:], start=False,
                         stop=True)
        out_sb = p_tinyb.tile([GPC, F], f32, tag="out_sb", name="out_sb")
        nc.scalar.copy(out_sb[:], out_ps[:])
        nc.sync.dma_start(out_d, out_sb[:])

    nc.compile()
    return nc


def _prep_consts(W1, b1, W2, b2, Wr, br):
    W1 = np.asarray(W1, np.float32)
    W2 = np.asarray(W2, np.float32)
    Wr = np.asarray(Wr, np.float32)
    b1 = np.asarray(b1, np.float32)
    b2 = np.asarray(b2, np.float32)
    br = np.asarray(br, np.float32)
    bf = ml_dtypes.bfloat16
    consts = {
        "cw1": np.ascontiguousarray(W1.astype(bf)),
        "cw2": np.ascontiguousarray(W2.astype(bf)),
        "cwrs": np.ascontiguousarray(Wr[:F] / N),  # fold mean's 1/N
        "cwrm": np.ascontiguousarray(Wr[F:]),
        # fold b2 through Wr into the final bias (both pools shift by b2)
        "cbr": (br + b2 @ Wr[:F] + b2 @ Wr[F:]).reshape(1, F)
            .astype(np.float32),
        "cones": np.ones((128, 128), bf),
        "cones32": np.ones((1, GPC), np.float32),
    }
    with_b1 = bool(np.any(b1))
    if with_b1:
        consts["cb1"] = np.tile(b1.reshape(1, F), (128, 4)).astype(bf)
    return consts, with_b1


def _make_in_maps(x, adj, consts):
    bf = ml_dtypes.bfloat16
    x = np.asarray(x, np.float32).astype(bf)
    adj = np.asarray(adj, np.float32)
    idx = np.arange(N)
    in_maps = []
    for c in range(NCORES):
        # partition-major layouts so DMA descriptors are 4KB-contiguous
        xs = x[c * GPC:(c + 1) * GPC].reshape(GPC, 2, 128, F) \
            .transpose(2, 0, 1, 3)
        asd = adj[c * GPC:(c + 1) * GPC].astype(bf)
        asd[:, idx, idx] = np.array(1.0, bf)  # DenseGCNConv self-loop diag
        # [group, g, t, p, n] -> [p, group, t, g, n]
        asd = asd.reshape(NGRP, AGSZ, 2, 128, N).transpose(3, 0, 2, 1, 4)
        m = {"xin": np.ascontiguousarray(xs),
             "adjin": np.ascontiguousarray(asd)}
        m.update(consts)
        in_maps.append(m)
    return in_maps


def kernel(x, adj, W1, b1, W2, b2, Wr, br):
    from concourse.bass_utils import run_bass_kernel_spmd

    consts, with_b1 = _prep_consts(W1, b1, W2, b2, Wr, br)

    key = ("v7", with_b1)
    if key not in _CACHE:
        _CACHE[key] = _build_program(with_b1)
    nc = _CACHE[key]

    in_maps = _make_in_maps(x, adj, consts)
    res = run_bass_kernel_spmd(nc, in_maps, core_ids=list(range(NCORES)))
    out = np.concatenate([res.results[c]["out"] for c in range(NCORES)],
                         axis=0)
    return out
